# revision 3
# baseline (speedup 1.0000x reference)
"""Trainium2 Bass kernel for nn_Aliformer (dense transformer w/ knowledge attention).

Math (reference, B=4 L=1024 DM=512 DF=1024 H=8 DK=128):
  v/k/q       = x @ {Wv,Wk,Wq}.T + b            (B,L,1024)
  k_fei/q_fei = x_knowledge @ {Wkk,Wkq}.T + b   (B,L,1024)
  q,k,qf,kf   = second linear (1024->1024), then torch-style reshape
                (B,L,1024)->(B,8,1024,128) WITHOUT transpose.
  att  = (q@k^T + qf@kf^T)/sqrt(256); score = softmax(att)
  out  = (score @ v-reshaped) -> (B,L,1024); final = out @ Wout.T + bout

Key structural fact: the no-transpose reshape means head h of batch b only
touches rows [h*128,(h+1)*128) of the flattened (4096, ·) activations, so the
whole network decomposes into 32 independent 128-row blocks. Each of the 8
cores processes 4 contiguous blocks (512 rows) with zero communication.

Within the attention of a block, row index l' = 8r + c (r = row-in-block,
c = feature chunk). We compute everything in the permuted order i' = c*128+r
(softmax is permutation-invariant over the full axis).

v2 (fp8 DoubleRow): every matmul that only feeds the softmax LOGITS runs in
fp8e4m3 with perf_mode=DoubleRow (2 fp8 K-slices packed per PE cell -> one
K=256 contraction per instruction):
  - stage-1 k/q/kf/qf projections (x quantized to fp8; the v projection keeps
    x in bf16 since v errors pass straight to the output),
  - all four stage-2 projections (t1 stored fp8),
  - the attention QK^T matmuls, pairing (k2,q2) with (kf2,qf2) so
    att+att_fei accumulate inside one DoubleRow instruction.
Score@V / fc_out / v stay bf16 (fp8 there fails the 2e-2 gate; emulated
end-to-end error of this split is 1.4e-2).

Stage-1 biases are folded into stage-2 on the host (b2' = b2 + W2 @ b1), so
stage-1 PSUM->SBUF moves are pure quantizing copies. Stage-2 outputs write
directly into paired attention layouts kk/qq[128d, block, {base,fei}, i'] so
the DoubleRow attention operands are natural 3D slices. Softmax column sums
accumulate in bf16 (DVE 2x mode); partition reduction via ones-matmul;
bcast recip via gpsimd; division folded into the attention-output move.
bout is added on the host; bv on device (DMA-broadcast tile).

Weights are host-prepacked: w1 fp8 [128, 2pair, 2, 1024], w2 fp8
[128, 4pair, 2, 1024] where element [p, pr, i, dc*128+m] = W.T[(2pr+i)*128+p,
dc*128+m]; wv/wout stay bf16.
"""

import sys

for _p in ("/opt/trn_rl_repo", "/root/.axon_site/_ro/trn_rl_repo"):
    if _p not in sys.path:
        sys.path.insert(0, _p)

import numpy as np

import concourse.bass as bass
import concourse.mybir as mybir
import concourse.tile as tile
from concourse import bacc
from concourse.bass_utils import run_bass_kernel_spmd

F32 = mybir.dt.float32
BF16 = mybir.dt.bfloat16
F8 = mybir.dt.float8e4
DR = mybir.MatmulPerfMode.DoubleRow
EXP = mybir.ActivationFunctionType.Exp
IDENT = mybir.ActivationFunctionType.Identity

N_CORES = 8
R = 512          # rows per core
NB = 4           # 128-row blocks per core
DIN = 512        # model dim (input of proj1, output of fc_out)
DF = 1024        # d_ff / attention total feature dim
CH = 8           # feature chunks of DF
DK = 128
SCALE = 1.0 / 16.0   # 1/sqrt(2*DK)

_CACHE = {}


def build(loop_n=1, mode='full'):
    nc = bacc.Bacc("TRN2", target_bir_lowering=False, debug=False)

    xT = nc.dram_tensor("xT", [DIN, R], BF16, kind="ExternalInput")
    xT8 = nc.dram_tensor("xT8", [DIN, R], F8, kind="ExternalInput")
    xkT8 = nc.dram_tensor("xkT8", [DIN, R], F8, kind="ExternalInput")
    wvt = nc.dram_tensor("wvt", [DIN, DF], BF16, kind="ExternalInput")
    w1_names = ["wkt8", "wqt8", "wkkt8", "wkqt8"]
    w1 = {n: nc.dram_tensor(n, [128, 2, 2, DF], F8, kind="ExternalInput")
          for n in w1_names}
    w2_names = ["wk2t8", "wq2t8", "wkft8", "wqft8"]
    w2 = {n: nc.dram_tensor(n, [128, 4, 2, DF], F8, kind="ExternalInput")
          for n in w2_names}
    woutt = nc.dram_tensor("woutt", [DF, DIN], BF16, kind="ExternalInput")
    ones = nc.dram_tensor("ones", [128, 128], BF16, kind="ExternalInput")
    bv = nc.dram_tensor("bv", [DF], F32, kind="ExternalInput")
    b2pack = nc.dram_tensor("b2pack", [128, 32], F32, kind="ExternalInput")
    out = nc.dram_tensor("out", [R, DIN], F32, kind="ExternalOutput")

    with tile.TileContext(nc) as tc:
        with (
            tc.tile_pool(name="xp", bufs=1) as xp,          # xT/x8/xk8
            tc.tile_pool(name="wvp", bufs=4) as wvp,        # wv chunks
            tc.tile_pool(name="w1p", bufs=4) as w1p,        # fp8 w1 packs
            tc.tile_pool(name="w2p", bufs=4) as w2p,        # fp8 w2 packs
            tc.tile_pool(name="wop", bufs=1) as wop,        # fc_out weights
            tc.tile_pool(name="t1p", bufs=2) as t1p,        # stage1 out fp8
            tc.tile_pool(name="t2p", bufs=2) as t2p,        # kk/qq fp8
            tc.tile_pool(name="vp", bufs=1) as vp,          # v bf16
            tc.tile_pool(name="ep", bufs=2) as ep,          # expT bf16
            tc.tile_pool(name="smp", bufs=1) as smp,        # softmax work
            tc.tile_pool(name="op", bufs=2) as op,          # outT/final
            tc.tile_pool(name="bp", bufs=1) as bp,          # biases
            tc.tile_pool(name="psA", bufs=3, space="PSUM") as psA,   # 3 banks
            tc.tile_pool(name="psB", bufs=2, space="PSUM") as psB,   # 4 banks
            tc.tile_pool(name="psD", bufs=1, space="PSUM") as psD,   # 1 bank
        ):
            from contextlib import nullcontext
            loop_ctx = tc.For_i(0, loop_n, 1) if loop_n > 1 else nullcontext()
            with loop_ctx:
                # ---- load inputs -------------------------------------------
                xt_sb = xp.tile([128, 4, R], BF16, tag="xt")
                x8_sb = xp.tile([128, 4, R], F8, tag="x8")
                xk8_sb = xp.tile([128, 4, R], F8, tag="xk8")
                LOAD = mode != "compute"

                def tok(out_ap, in_row):
                    # token DMA: writes the tile (allocates its slot) cheaply
                    nc.sync.dma_start(out_ap, in_row)

                # critical path first: x chunk 0 + v-weights, then rest of x
                if LOAD:
                    nc.gpsimd.dma_start(xt_sb[:, 0, :], xT[0:128, :])
                else:
                    nc.gpsimd.dma_start(xt_sb[0:1, 0, 0:8], xT[0:1, 0:8])
                wv_c = []
                for kc in range(4):
                    t = wvp.tile([128, DF], BF16, tag="wv")
                    if LOAD:
                        nc.sync.dma_start(t[:], wvt[kc * 128:(kc + 1) * 128, :])
                    else:
                        tok(t[0:1, 0:8], wvt[0:1, 0:8])
                    wv_c.append(t)
                if LOAD:
                    for kc in range(1, 4):
                        nc.gpsimd.dma_start(xt_sb[:, kc, :],
                                            xT[kc * 128:(kc + 1) * 128, :])
                    for kc in range(4):
                        nc.gpsimd.dma_start(x8_sb[:, kc, :],
                                            xT8[kc * 128:(kc + 1) * 128, :])
                else:
                    for kc in range(1, 4):
                        nc.gpsimd.dma_start(xt_sb[0:1, kc, 0:8], xT[0:1, 0:8])
                    for kc in range(4):
                        nc.gpsimd.dma_start(x8_sb[0:1, kc, 0:8], xT8[0:1, 0:8])

                # bv broadcast to all partitions for the row-major v layout
                bvb = bp.tile([128, DF], F32, tag="bvb")
                if LOAD:
                    nc.gpsimd.dma_start(
                        bvb[:], bass.AP(bv, 0, [[0, 128], [1, DF]]))
                else:
                    nc.gpsimd.dma_start(bvb[0:1, 0:8], bv[0:8])
                # packed folded stage2 biases: [:, i*8+dc] = b2'_i[dc*128+p]
                b2p_sb = bp.tile([128, 32], F32, tag="b2p")
                if LOAD:
                    nc.gpsimd.dma_start(b2p_sb[:], b2pack[:])
                else:
                    nc.gpsimd.dma_start(b2p_sb[0:1, 0:8], b2pack[0:1, 0:8])
                ones_sb = bp.tile([128, 128], BF16, tag="ones")
                nc.sync.dma_start(ones_sb[:], ones[:])
                # PE warmup: matmuls on the first-arrived x chunk fill the
                # head weight-DMA wait and warm the HAM clock gate
                warm_ps = psB.tile([128, 1024], F32, tag="attp")
                for i in range(16):
                    nc.tensor.matmul(warm_ps[:, 0:512], xt_sb[:, 0, 0:128],
                                     xt_sb[:, 0, :], start=True, stop=True)
                if LOAD:
                    for kc in range(4):
                        nc.gpsimd.dma_start(xk8_sb[:, kc, :],
                                            xkT8[kc * 128:(kc + 1) * 128, :])
                else:
                    for kc in range(4):
                        nc.gpsimd.dma_start(xk8_sb[0:1, kc, 0:8],
                                            xkT8[0:1, 0:8])

                # ---- v = x @ Wv.T + bv (bf16, row-major 128 x 1024) --------
                v_sb = vp.tile([128, NB * DF], BF16, tag="v")
                for rt in range(NB):
                    for n in range(2):
                        ps = psA.tile([128, 512], F32, tag="psA")
                        for kc in range(4):
                            nc.tensor.matmul(
                                ps[:],
                                xt_sb[:, kc, rt * 128:(rt + 1) * 128],
                                wv_c[kc][:, n * 512:(n + 1) * 512],
                                start=(kc == 0), stop=(kc == 3))
                        nc.vector.tensor_add(
                            v_sb[:, rt * DF + n * 512: rt * DF + (n + 1) * 512],
                            ps[:], bvb[:, n * 512:(n + 1) * 512])

                # paired attention-layout tiles: [128 d, block, {base,fei}, i']
                kk = t2p.tile([128, NB, 2, DF], F8, tag="kk")
                qq = t2p.tile([128, NB, 2, DF], F8, tag="qq")

                # ---- k/q/kf/qf: fp8 DoubleRow two-stage projections --------
                def stage1(x8src, w1n):
                    w1_sb = w1p.tile([128, 2, 2, DF], F8, tag="w1")
                    if LOAD:
                        nc.sync.dma_start(w1_sb[:], w1[w1n][:])
                    else:
                        tok(w1_sb[0:1, 0, 0, 0:8], w1[w1n][0:1, 0, 0, 0:8])
                    t1 = t1p.tile([128, CH, R], F8, tag="t1")
                    for dc in range(CH):
                        ps = psA.tile([128, 512], F32, tag="psA")
                        for pr in range(2):
                            nc.tensor.matmul(
                                ps[:], w1_sb[:, pr, :, dc * 128:(dc + 1) * 128],
                                x8src[:, 2 * pr:2 * pr + 2, :],
                                start=(pr == 0), stop=(pr == 1), perf_mode=DR)
                        # bias folded into stage2: pure quantizing copy
                        if dc % 2 == 0:
                            nc.scalar.activation(t1[:, dc, :], ps[:], IDENT)
                        else:
                            nc.vector.tensor_copy(t1[:, dc, :], ps[:])
                    return t1

                def stage2(dst, iq, bidx, w2n, t1):
                    w2_sb = w2p.tile([128, 4, 2, DF], F8, tag="w2")
                    if LOAD:
                        nc.sync.dma_start(w2_sb[:], w2[w2n][:])
                    else:
                        tok(w2_sb[0:1, 0, 0, 0:8], w2[w2n][0:1, 0, 0, 0:8])
                    for dc in range(CH):
                        ps = psA.tile([128, 512], F32, tag="psA")
                        for pr in range(4):
                            nc.tensor.matmul(
                                ps[:], w2_sb[:, pr, :, dc * 128:(dc + 1) * 128],
                                t1[:, 2 * pr:2 * pr + 2, :],
                                start=(pr == 0), stop=(pr == 3), perf_mode=DR)
                        out_ap = dst[:, :, iq, dc * 128:(dc + 1) * 128]
                        bcol = b2p_sb[:, bidx * 8 + dc:bidx * 8 + dc + 1]
                        if dc % 2 == 0:
                            nc.vector.tensor_scalar_add(out_ap, ps[:], bcol)
                        else:
                            nc.scalar.activation(out_ap, ps[:], IDENT,
                                                 bias=bcol)

                # paired emission (s1a, s1b, s2a, s2b) for write->read slack
                for (xa, w1a, dsta, iqa, bia, w2a), (xb, w1b, dstb, iqb, bib, w2b) in [
                    ((x8_sb, "wkt8", kk, 0, 0, "wk2t8"),
                     (x8_sb, "wqt8", qq, 0, 1, "wq2t8")),
                    ((xk8_sb, "wkkt8", kk, 1, 2, "wkft8"),
                     (xk8_sb, "wkqt8", qq, 1, 3, "wqft8")),
                ]:
                    t1a = stage1(xa, w1a)
                    t1b = stage1(xb, w1b)
                    stage2(dsta, iqa, bia, w2a, t1a)
                    stage2(dstb, iqb, bib, w2b, t1b)

                # fc_out weights (needed from the first block's tail onward)
                wo_sb = wop.tile([128, CH, DIN], BF16, tag="wo")
                if LOAD:
                    nc.sync.dma_start(
                        wo_sb[:], woutt.rearrange("(c p) j -> p c j", p=128))
                else:
                    nc.sync.dma_start(wo_sb[0:1, 0, 0:8], woutt[0:1, 0:8])

                # ---- attention + fc_out, software-pipelined over blocks ----
                # Engine queues are in-order FIFOs, so emission order = PE
                # order. ACT's exp cadence (~1us/chunk) paces the att matmuls
                # via PSUM-bank recycling, so interleave block b's att mms
                # with block b-1's score@v/fc_out mms: the PE does useful
                # work during every exp wait instead of micro-idling (which
                # would also oscillate the HAM clock gate on HW).
                exp_t = {}
                bcast_t = {}

                def sum_block(b, sumc):
                    # partition reduce via ones-matmul (DVE can't cross
                    # partitions); 2 tiny PE matmuls
                    recip = smp.tile([1, 1024], F32, tag="recip")
                    for nh in range(2):
                        cs = psD.tile([1, 512], F32, tag="cs")
                        nc.tensor.matmul(cs[:], ones_sb[:, 0:1],
                                         sumc[:, nh * 512:(nh + 1) * 512],
                                         start=True, stop=True)
                        nc.vector.reciprocal(
                            recip[0:1, nh * 512:(nh + 1) * 512], cs[:])
                    bcastR = smp.tile([128, 1024], F32, tag="bcastR")
                    nc.gpsimd.partition_broadcast(bcastR[:], recip[:])
                    bcast_t[b] = bcastR

                def att_out_block(b):
                    """Emit att(b) interleaved with the out-path of b-1."""
                    ob = b - 1
                    if b < NB:
                        expT = ep.tile([128, CH, 1024], BF16, tag="expT")
                        exp_t[b] = expT
                        sumc = smp.tile([128, 1024], BF16, tag="sumc")
                    if ob >= 0:
                        expO = exp_t.pop(ob)
                        outp0 = psA.tile([128, 512], F32, tag="psA")
                        outp1 = psA.tile([128, 512], F32, tag="psA")
                        outp = [outp0, outp1]
                    for cj in range(CH):
                        if b < NB:
                            attp = psB.tile([128, 1024], F32, tag="attp")
                            for nh in range(2):
                                # one DoubleRow mm = q.k + qf.kf (K=256)
                                nc.tensor.matmul(
                                    attp[:, nh * 512:(nh + 1) * 512],
                                    kk[:, b, :, cj * 128:(cj + 1) * 128],
                                    qq[:, b, :, nh * 512:(nh + 1) * 512],
                                    start=True, stop=True, perf_mode=DR)
                            # exp((att+att_fei)/16): PSUM -> SBUF on ACT
                            nc.scalar.activation(expT[:, cj, :], attp[:], EXP,
                                                 scale=SCALE)
                            # bf16 column-sum accumulation on DVE (2x mode)
                            if cj == 1:
                                nc.vector.tensor_add(sumc[:], expT[:, 0, :],
                                                     expT[:, 1, :])
                            elif cj > 1:
                                nc.vector.tensor_add(sumc[:], sumc[:],
                                                     expT[:, cj, :])
                        if ob >= 0:
                            # score@v for block b-1, chunk cj (dep-free: its
                            # exps finished a block ago)
                            for nh in range(2):
                                nc.tensor.matmul(
                                    outp[nh][:],
                                    v_sb[:, ob * DF + cj * 128:
                                         ob * DF + (cj + 1) * 128],
                                    expO[:, cj, nh * 512:(nh + 1) * 512],
                                    start=(cj == 0), stop=(cj == 7))
                    if ob >= 0:
                        bs = ob * 128
                        bcastR = bcast_t.pop(ob)
                        outT = op.tile([128, 1024], BF16, tag="outT")
                        for nh in range(2):
                            nc.vector.tensor_mul(
                                outT[:, nh * 512:(nh + 1) * 512], outp[nh][:],
                                bcastR[:, nh * 512:(nh + 1) * 512])
                        # fc_out for this block: (128 rows, 512 dm)
                        fcp = psA.tile([128, 512], F32, tag="psA")
                        for c in range(CH):
                            nc.tensor.matmul(fcp[:],
                                             outT[:, c * 128:(c + 1) * 128],
                                             wo_sb[:, c, :],
                                             start=(c == 0), stop=(c == 7))
                        final = op.tile([128, 512], F32, tag="final")
                        nc.vector.tensor_copy(final[:], fcp[:])
                        nc.gpsimd.dma_start(out[bs:bs + 128, :], final[:])
                    if b < NB:
                        sum_block(b, sumc)

                for b in range(NB + 1):
                    att_out_block(b)

    nc.compile()
    return nc


def build_dma(loop_n=1):
    """DMA-only probe: all input loads + output stores, no compute."""
    nc = bacc.Bacc("TRN2", target_bir_lowering=False, debug=False)
    xT = nc.dram_tensor("xT", [DIN, R], BF16, kind="ExternalInput")
    xT8 = nc.dram_tensor("xT8", [DIN, R], F8, kind="ExternalInput")
    xkT8 = nc.dram_tensor("xkT8", [DIN, R], F8, kind="ExternalInput")
    wvt = nc.dram_tensor("wvt", [DIN, DF], BF16, kind="ExternalInput")
    w1_names = ["wkt8", "wqt8", "wkkt8", "wkqt8"]
    w1 = {n: nc.dram_tensor(n, [128, 2, 2, DF], F8, kind="ExternalInput")
          for n in w1_names}
    w2_names = ["wk2t8", "wq2t8", "wkft8", "wqft8"]
    w2 = {n: nc.dram_tensor(n, [128, 4, 2, DF], F8, kind="ExternalInput")
          for n in w2_names}
    woutt = nc.dram_tensor("woutt", [DF, DIN], BF16, kind="ExternalInput")
    ones = nc.dram_tensor("ones", [128, 128], BF16, kind="ExternalInput")
    bv = nc.dram_tensor("bv", [DF], F32, kind="ExternalInput")
    b2pack = nc.dram_tensor("b2pack", [128, 32], F32, kind="ExternalInput")
    out = nc.dram_tensor("out", [R, DIN], F32, kind="ExternalOutput")
    with tile.TileContext(nc) as tc:
        with (
            tc.tile_pool(name="xp", bufs=1) as xp,
            tc.tile_pool(name="wvp", bufs=4) as wvp,
            tc.tile_pool(name="w1p", bufs=4) as w1p,
            tc.tile_pool(name="w2p", bufs=4) as w2p,
            tc.tile_pool(name="wop", bufs=1) as wop,
            tc.tile_pool(name="bp", bufs=1) as bp,
            tc.tile_pool(name="op", bufs=1) as op,
        ):
            from contextlib import nullcontext
            loop_ctx = tc.For_i(0, loop_n, 1) if loop_n > 1 else nullcontext()
            with loop_ctx:
                xt_sb = xp.tile([128, 4, R], BF16, tag="xt")
                x8_sb = xp.tile([128, 4, R], F8, tag="x8")
                xk8_sb = xp.tile([128, 4, R], F8, tag="xk8")
                for kc in range(4):
                    nc.sync.dma_start(xt_sb[:, kc, :],
                                      xT[kc * 128:(kc + 1) * 128, :])
                    nc.sync.dma_start(x8_sb[:, kc, :],
                                      xT8[kc * 128:(kc + 1) * 128, :])
                    nc.sync.dma_start(xk8_sb[:, kc, :],
                                      xkT8[kc * 128:(kc + 1) * 128, :])
                for kc in range(4):
                    t = wvp.tile([128, DF], BF16, tag="wv")
                    nc.sync.dma_start(t[:], wvt[kc * 128:(kc + 1) * 128, :])
                for n in w1_names:
                    t = w1p.tile([128, 2, 2, DF], F8, tag="w1")
                    nc.sync.dma_start(t[:], w1[n][:])
                for n in w2_names:
                    t = w2p.tile([128, 4, 2, DF], F8, tag="w2")
                    nc.sync.dma_start(t[:], w2[n][:])
                wo_d = wop.tile([128, CH, DIN], BF16, tag="wo")
                nc.sync.dma_start(
                    wo_d[:], woutt.rearrange("(c p) j -> p c j", p=128))
                bvb = bp.tile([128, DF], F32, tag="bvb")
                nc.sync.dma_start(bvb[:], bass.AP(bv, 0, [[0, 128], [1, DF]]))
                b2p_sb = bp.tile([128, 32], F32, tag="b2p")
                nc.sync.dma_start(b2p_sb[:], b2pack[:])
                ones_sb = bp.tile([128, 128], BF16, tag="ones")
                nc.sync.dma_start(ones_sb[:], ones[:])
                fin_d = op.tile([128, 512], F32, tag="final")
                nc.gpsimd.memset(fin_d[:], 0.0)
                for b in range(NB):
                    nc.sync.dma_start(out[b * 128:(b + 1) * 128, :], fin_d[:])
    nc.compile()
    return nc


def prep_in_maps(inputs):
    import ml_dtypes
    NPBF = ml_dtypes.bfloat16
    NPF8 = ml_dtypes.float8_e4m3

    x = np.ascontiguousarray(inputs["x"], dtype=np.float32)
    xk = np.ascontiguousarray(inputs["x_knowledge"], dtype=np.float32)
    B, L, DM = x.shape
    x_flat = x.reshape(B * L, DM)
    xk_flat = xk.reshape(B * L, DM)
    f32 = np.float32

    def pack_w(name, npairs):
        # [p, pr, i, m] = W.T[(2*pr+i)*128+p, m], fp8
        WT = np.ascontiguousarray(np.asarray(inputs[name], f32).T)
        arr = WT.reshape(npairs, 2, 128, DF).transpose(2, 0, 1, 3)
        return np.ascontiguousarray(arr).astype(NPF8)

    def fold_b2(w2n, b2n, b1n):
        return (np.asarray(inputs[b2n], f32)
                + np.asarray(inputs[w2n], f32) @ np.asarray(inputs[b1n], f32))

    b2pack = np.zeros((128, 32), dtype=f32)
    for i, (w2n, b2n, b1n) in enumerate([
        ("Wk2", "bk2", "bk"), ("Wq2", "bq2", "bq"),
        ("Wkf", "bkf", "bkk"), ("Wqf", "bqf", "bkq"),
    ]):
        b2pack[:, i * 8:(i + 1) * 8] = fold_b2(w2n, b2n, b1n).reshape(8, 128).T

    shared = {
        "wvt": np.ascontiguousarray(np.asarray(inputs["Wv"], f32).T).astype(NPBF),
        "wkt8": pack_w("Wk", 2), "wqt8": pack_w("Wq", 2),
        "wkkt8": pack_w("Wkk", 2), "wkqt8": pack_w("Wkq", 2),
        "wk2t8": pack_w("Wk2", 4), "wq2t8": pack_w("Wq2", 4),
        "wkft8": pack_w("Wkf", 4), "wqft8": pack_w("Wqf", 4),
        "woutt": np.ascontiguousarray(np.asarray(inputs["Wout"], f32).T).astype(NPBF),
        "ones": np.ones((128, 128), dtype=NPBF),
        "bv": np.asarray(inputs["bv"], dtype=f32),
        "b2pack": b2pack,
    }
    in_maps = []
    for c in range(N_CORES):
        sl = slice(c * R, (c + 1) * R)
        m = dict(shared)
        xTc = np.ascontiguousarray(x_flat[sl].T)
        xkTc = np.ascontiguousarray(xk_flat[sl].T)
        m["xT"] = xTc.astype(NPBF)
        m["xT8"] = xTc.astype(NPF8)
        m["xkT8"] = xkTc.astype(NPF8)
        in_maps.append(m)
    return in_maps


def kernel(**inputs):
    if "nc" not in _CACHE:
        _CACHE["nc"] = build()
    nc = _CACHE["nc"]
    in_maps = prep_in_maps(inputs)
    B, L, DM = inputs["x"].shape
    f32 = np.float32

    res = run_bass_kernel_spmd(nc, in_maps, core_ids=list(range(N_CORES)))
    _CACHE["last_results"] = res
    out_flat = np.concatenate([res.results[c]["out"] for c in range(N_CORES)],
                              axis=0)
    out_flat = out_flat + np.asarray(inputs["bout"], dtype=f32)[None, :]
    return out_flat.reshape(B, L, DM).astype(np.float32)


if __name__ == "__main__":
    if "--compile-only" in sys.argv:
        import tempfile
        from concourse.bass_utils import compile_bass_kernel
        nc = build()
        print("bacc build OK; walrus-compiling...")
        print("OK:", compile_bass_kernel(nc, tempfile.mkdtemp()))


# revision 20
# speedup vs baseline: 1.0250x; 1.0250x over previous
"""Trainium2 Bass kernel for nn_Aliformer (dense transformer w/ knowledge attention).

Math (reference, B=4 L=1024 DM=512 DF=1024 H=8 DK=128):
  v/k/q       = x @ {Wv,Wk,Wq}.T + b            (B,L,1024)
  k_fei/q_fei = x_knowledge @ {Wkk,Wkq}.T + b   (B,L,1024)
  q,k,qf,kf   = second linear (1024->1024), then torch-style reshape
                (B,L,1024)->(B,8,1024,128) WITHOUT transpose.
  att  = (q@k^T + qf@kf^T)/sqrt(256); score = softmax(att)
  out  = (score @ v-reshaped) -> (B,L,1024); final = out @ Wout.T + bout

Key structural fact: the no-transpose reshape means head h of batch b only
touches rows [h*128,(h+1)*128) of the flattened (4096, ·) activations, so the
whole network decomposes into 32 independent 128-row blocks. Each of the 8
cores processes 4 contiguous blocks (512 rows) with zero communication.

Within the attention of a block, row index l' = 8r + c (r = row-in-block,
c = feature chunk). We compute everything in the permuted order i' = c*128+r
(softmax is permutation-invariant over the full axis).

v2 (fp8 DoubleRow): every matmul that only feeds the softmax LOGITS runs in
fp8e4m3 with perf_mode=DoubleRow (2 fp8 K-slices packed per PE cell -> one
K=256 contraction per instruction):
  - stage-1 k/q/kf/qf projections (x quantized to fp8; the v projection keeps
    x in bf16 since v errors pass straight to the output),
  - all four stage-2 projections (t1 stored fp8),
  - the attention QK^T matmuls, pairing (k2,q2) with (kf2,qf2) so
    att+att_fei accumulate inside one DoubleRow instruction.
Score@V / fc_out / v stay bf16 (fp8 there fails the 2e-2 gate; emulated
end-to-end error of this split is 1.4e-2).

Stage-1 biases are folded into stage-2 on the host (b2' = b2 + W2 @ b1), so
stage-1 PSUM->SBUF moves are pure quantizing copies. Stage-2 outputs write
directly into paired attention layouts kk/qq[128d, block, {base,fei}, i'] so
the DoubleRow attention operands are natural 3D slices. Softmax column sums
accumulate in bf16 (DVE 2x mode); partition reduction via ones-matmul;
bcast recip via gpsimd; division folded into the attention-output move.
bout is added on the host; bv on device (DMA-broadcast tile).

Weights are host-prepacked: w1 fp8 [128, 2pair, 2, 1024], w2 fp8
[128, 4pair, 2, 1024] where element [p, pr, i, dc*128+m] = W.T[(2pr+i)*128+p,
dc*128+m]; wv/wout stay bf16.
"""

import sys

for _p in ("/opt/trn_rl_repo", "/root/.axon_site/_ro/trn_rl_repo"):
    if _p not in sys.path:
        sys.path.insert(0, _p)

import numpy as np

import concourse.bass as bass
import concourse.mybir as mybir
import concourse.tile as tile
from concourse import bacc
from concourse.bass_utils import run_bass_kernel_spmd

F32 = mybir.dt.float32
BF16 = mybir.dt.bfloat16
F8 = mybir.dt.float8e4
DR = mybir.MatmulPerfMode.DoubleRow
EXP = mybir.ActivationFunctionType.Exp
IDENT = mybir.ActivationFunctionType.Identity

N_CORES = 8
R = 512          # rows per core
NB = 4           # 128-row blocks per core
DIN = 512        # model dim (input of proj1, output of fc_out)
DF = 1024        # d_ff / attention total feature dim
CH = 8           # feature chunks of DF
DK = 128
SCALE = 1.0 / 16.0   # 1/sqrt(2*DK)

_CACHE = {}


def build(loop_n=1, mode='full'):
    nc = bacc.Bacc("TRN2", target_bir_lowering=False, debug=False)

    xT = nc.dram_tensor("xT", [DIN, R], BF16, kind="ExternalInput")
    xT8 = nc.dram_tensor("xT8", [DIN, R], F8, kind="ExternalInput")
    xkT8 = nc.dram_tensor("xkT8", [DIN, R], F8, kind="ExternalInput")
    wvt = nc.dram_tensor("wvt", [DIN, DF], BF16, kind="ExternalInput")
    w1_names = ["wkt8", "wqt8", "wkkt8", "wkqt8"]
    w1 = {n: nc.dram_tensor(n, [128, 2, 2, DF], F8, kind="ExternalInput")
          for n in w1_names}
    w2_names = ["wk2t8", "wq2t8", "wkft8", "wqft8"]
    w2 = {n: nc.dram_tensor(n, [128, 4, 2, DF], F8, kind="ExternalInput")
          for n in w2_names}
    woutt = nc.dram_tensor("woutt", [DF, DIN], BF16, kind="ExternalInput")
    ones = nc.dram_tensor("ones", [128, 128], BF16, kind="ExternalInput")
    bv = nc.dram_tensor("bv", [DF], F32, kind="ExternalInput")
    b2pack = nc.dram_tensor("b2pack", [128, 32], F32, kind="ExternalInput")
    out = nc.dram_tensor("out", [R, DIN], F32, kind="ExternalOutput")

    with tile.TileContext(nc) as tc:
        with (
            tc.tile_pool(name="xp", bufs=1) as xp,          # xT/x8/xk8
            tc.tile_pool(name="wvp", bufs=4) as wvp,        # wv chunks
            tc.tile_pool(name="w1p", bufs=4) as w1p,        # fp8 w1 packs
            tc.tile_pool(name="w2p", bufs=4) as w2p,        # fp8 w2 packs
            tc.tile_pool(name="wop", bufs=1) as wop,        # fc_out weights
            tc.tile_pool(name="t1p", bufs=2) as t1p,        # stage1 out fp8
            tc.tile_pool(name="t2p", bufs=2) as t2p,        # kk/qq fp8
            tc.tile_pool(name="vp", bufs=1) as vp,          # v bf16
            tc.tile_pool(name="ep", bufs=2) as ep,          # expT bf16
            tc.tile_pool(name="smp", bufs=1) as smp,        # softmax work
            tc.tile_pool(name="op", bufs=2) as op,          # outT/final
            tc.tile_pool(name="bp", bufs=1) as bp,          # biases
            tc.tile_pool(name="psA", bufs=3, space="PSUM") as psA,   # 3 banks
            tc.tile_pool(name="psB", bufs=2, space="PSUM") as psB,   # 4 banks
            tc.tile_pool(name="psD", bufs=1, space="PSUM") as psD,   # 1 bank
        ):
            from contextlib import nullcontext
            loop_ctx = tc.For_i(0, loop_n, 1) if loop_n > 1 else nullcontext()
            with loop_ctx:
                # ---- load inputs -------------------------------------------
                xt_sb = xp.tile([128, 4, R], BF16, tag="xt")
                x8_sb = xp.tile([128, 4, R], F8, tag="x8")
                xk8_sb = xp.tile([128, 4, R], F8, tag="xk8")
                LOAD = mode != "compute"

                def tok(out_ap, in_row):
                    # token DMA: writes the tile (allocates its slot) cheaply
                    nc.sync.dma_start(out_ap, in_row)

                ATTONLY = mode == "attonly"
                PROJ = mode == "proj"
                # critical path first: x chunk 0 + v-weights, then rest of x
                if LOAD:
                    nc.gpsimd.dma_start(xt_sb[:, 0, :], xT[0:128, :])
                else:
                    nc.gpsimd.dma_start(xt_sb[0:1, 0, 0:8], xT[0:1, 0:8])
                wv_c = []
                for kc in range(4):
                    t = wvp.tile([128, DF], BF16, tag="wv")
                    if LOAD:
                        nc.sync.dma_start(t[:], wvt[kc * 128:(kc + 1) * 128, :])
                    else:
                        tok(t[0:1, 0:8], wvt[0:1, 0:8])
                    wv_c.append(t)
                if LOAD:
                    for kc in range(1, 4):
                        nc.gpsimd.dma_start(xt_sb[:, kc, :],
                                            xT[kc * 128:(kc + 1) * 128, :])
                    for kc in range(4):
                        nc.gpsimd.dma_start(x8_sb[:, kc, :],
                                            xT8[kc * 128:(kc + 1) * 128, :])
                else:
                    for kc in range(1, 4):
                        nc.gpsimd.dma_start(xt_sb[0:1, kc, 0:8], xT[0:1, 0:8])
                    for kc in range(4):
                        nc.gpsimd.dma_start(x8_sb[0:1, kc, 0:8], xT8[0:1, 0:8])

                # bv broadcast to all partitions for the row-major v layout
                bvb = bp.tile([128, DF], F32, tag="bvb")
                if LOAD:
                    nc.gpsimd.dma_start(
                        bvb[:], bass.AP(bv, 0, [[0, 128], [1, DF]]))
                else:
                    nc.gpsimd.dma_start(bvb[0:1, 0:8], bv[0:8])
                # packed folded stage2 biases: [:, i*8+dc] = b2'_i[dc*128+p]
                b2p_sb = bp.tile([128, 32], F32, tag="b2p")
                if LOAD:
                    nc.gpsimd.dma_start(b2p_sb[:], b2pack[:])
                else:
                    nc.gpsimd.dma_start(b2p_sb[0:1, 0:8], b2pack[0:1, 0:8])
                ones_sb = bp.tile([128, 128], BF16, tag="ones")
                nc.sync.dma_start(ones_sb[:], ones[:])
                # PE warmup: matmuls on the first-arrived x chunk fill the
                # head weight-DMA wait and warm the HAM clock gate
                warm_ps = psB.tile([128, 1024], F32, tag="attp")
                for i in range(8):
                    nc.tensor.matmul(warm_ps[:, 0:512], xt_sb[:, 0, 0:128],
                                     xt_sb[:, 0, :], start=True, stop=True)
                if LOAD:
                    for kc in range(4):
                        nc.gpsimd.dma_start(xk8_sb[:, kc, :],
                                            xkT8[kc * 128:(kc + 1) * 128, :])
                else:
                    for kc in range(4):
                        nc.gpsimd.dma_start(xk8_sb[0:1, kc, 0:8],
                                            xkT8[0:1, 0:8])

                # ---- v = x @ Wv.T + bv (bf16, row-major 128 x 1024) --------
                v_sb = vp.tile([128, NB * DF], BF16, tag="v")
                if mode != "attonly":
                    for rt in range(NB):
                        for n in range(2):
                            ps = psA.tile([128, 512], F32, tag="psA")
                            for kc in range(4):
                                nc.tensor.matmul(
                                    ps[:],
                                    xt_sb[:, kc, rt * 128:(rt + 1) * 128],
                                    wv_c[kc][:, n * 512:(n + 1) * 512],
                                    start=(kc == 0), stop=(kc == 3))
                            nc.vector.tensor_add(
                                v_sb[:, rt * DF + n * 512:
                                     rt * DF + (n + 1) * 512],
                                ps[:], bvb[:, n * 512:(n + 1) * 512])

                # paired attention-layout tiles: [128 d, block, {base,fei}, i']
                kk = t2p.tile([128, NB, 2, DF], F8, tag="kk")
                qq = t2p.tile([128, NB, 2, DF], F8, tag="qq")
                if ATTONLY:
                    # stage kk/qq/v from same-shaped weight tensors (finite
                    # junk values) to time the att/out phase in isolation
                    nc.sync.dma_start(kk[:], w2["wk2t8"][:])
                    nc.sync.dma_start(qq[:], w2["wq2t8"][:])
                    nc.sync.dma_start(
                        v_sb[:].bitcast(BF16),
                        woutt.rearrange("(c p) j -> p (c j)", p=128))

                # ---- k/q/kf/qf: fp8 DoubleRow two-stage projections --------
                def stage1(x8src, w1n):
                    w1_sb = w1p.tile([128, 2, 2, DF], F8, tag="w1")
                    if LOAD:
                        nc.sync.dma_start(w1_sb[:], w1[w1n][:])
                    else:
                        tok(w1_sb[0:1, 0, 0, 0:8], w1[w1n][0:1, 0, 0, 0:8])
                    t1 = t1p.tile([128, CH, R], F8, tag="t1")
                    for dc in range(CH):
                        ps = psA.tile([128, 512], F32, tag="psA")
                        for pr in range(2):
                            nc.tensor.matmul(
                                ps[:], w1_sb[:, pr, :, dc * 128:(dc + 1) * 128],
                                x8src[:, 2 * pr:2 * pr + 2, :],
                                start=(pr == 0), stop=(pr == 1), perf_mode=DR)
                        # bias folded into stage2: pure quantizing copy
                        if dc % 2 == 0:
                            nc.scalar.activation(t1[:, dc, :], ps[:], IDENT)
                        else:
                            nc.vector.tensor_copy(t1[:, dc, :], ps[:])
                    return t1

                def stage2(dst, iq, bidx, w2n, t1):
                    w2_sb = w2p.tile([128, 4, 2, DF], F8, tag="w2")
                    if LOAD:
                        nc.sync.dma_start(w2_sb[:], w2[w2n][:])
                    else:
                        tok(w2_sb[0:1, 0, 0, 0:8], w2[w2n][0:1, 0, 0, 0:8])
                    for dc in range(CH):
                        ps = psA.tile([128, 512], F32, tag="psA")
                        for pr in range(4):
                            nc.tensor.matmul(
                                ps[:], w2_sb[:, pr, :, dc * 128:(dc + 1) * 128],
                                t1[:, 2 * pr:2 * pr + 2, :],
                                start=(pr == 0), stop=(pr == 3), perf_mode=DR)
                        out_ap = dst[:, :, iq, dc * 128:(dc + 1) * 128]
                        bcol = b2p_sb[:, bidx * 8 + dc:bidx * 8 + dc + 1]
                        if dc % 2 == 0:
                            nc.vector.tensor_scalar_add(out_ap, ps[:], bcol)
                        else:
                            nc.scalar.activation(out_ap, ps[:], IDENT,
                                                 bias=bcol)

                # paired emission (s1a, s1b, s2a, s2b) for write->read slack
                if not ATTONLY:
                    for (xa, w1a, dsta, iqa, bia, w2a), (xb, w1b, dstb, iqb, bib, w2b) in [
                        ((x8_sb, "wkt8", kk, 0, 0, "wk2t8"),
                         (x8_sb, "wqt8", qq, 0, 1, "wq2t8")),
                        ((xk8_sb, "wkkt8", kk, 1, 2, "wkft8"),
                         (xk8_sb, "wkqt8", qq, 1, 3, "wqft8")),
                    ]:
                        t1a = stage1(xa, w1a)
                        t1b = stage1(xb, w1b)
                        stage2(dsta, iqa, bia, w2a, t1a)
                        stage2(dstb, iqb, bib, w2b, t1b)

                # fc_out weights (needed from the first block's tail onward)
                wo_sb = wop.tile([128, CH, DIN], BF16, tag="wo")
                if LOAD:
                    nc.sync.dma_start(
                        wo_sb[:], woutt.rearrange("(c p) j -> p c j", p=128))
                else:
                    nc.sync.dma_start(wo_sb[0:1, 0, 0:8], woutt[0:1, 0:8])

                # ---- attention + fc_out, software-pipelined over blocks ----
                # Engine queues are in-order FIFOs, so emission order = PE
                # order. ACT's exp cadence (~1us/chunk) paces the att matmuls
                # via PSUM-bank recycling, so interleave block b's att mms
                # with block b-1's score@v/fc_out mms: the PE does useful
                # work during every exp wait instead of micro-idling (which
                # would also oscillate the HAM clock gate on HW).
                exp_t = {}
                bcast_t = {}

                def sum_block(b, sumc):
                    # partition reduce via ones-matmul (DVE can't cross
                    # partitions); 2 tiny PE matmuls
                    recip = smp.tile([1, 1024], F32, tag="recip")
                    for nh in range(2):
                        cs = psD.tile([1, 512], F32, tag="cs")
                        nc.tensor.matmul(cs[:], ones_sb[:, 0:1],
                                         sumc[:, nh * 512:(nh + 1) * 512],
                                         start=True, stop=True)
                        nc.vector.reciprocal(
                            recip[0:1, nh * 512:(nh + 1) * 512], cs[:])
                    bcastR = smp.tile([128, 1024], F32, tag="bcastR")
                    nc.gpsimd.partition_broadcast(bcastR[:], recip[:])
                    bcast_t[b] = bcastR

                def att_out_block(b):
                    """Emit att(b) interleaved with the out-path of b-1."""
                    ob = b - 1
                    if b < NB:
                        expT = ep.tile([128, CH, 1024], BF16, tag="expT")
                        exp_t[b] = expT
                        sumc = smp.tile([128, 1024], BF16, tag="sumc")
                    if ob >= 0:
                        expO = exp_t.pop(ob)
                        outp0 = psA.tile([128, 512], F32, tag="psA")
                        outp1 = psA.tile([128, 512], F32, tag="psA")
                        outp = [outp0, outp1]
                    for cj in range(CH):
                        if b < NB:
                            attp = psB.tile([128, 1024], F32, tag="attp")
                            for nh in range(2):
                                # one DoubleRow mm = q.k + qf.kf (K=256)
                                nc.tensor.matmul(
                                    attp[:, nh * 512:(nh + 1) * 512],
                                    kk[:, b, :, cj * 128:(cj + 1) * 128],
                                    qq[:, b, :, nh * 512:(nh + 1) * 512],
                                    start=True, stop=True, perf_mode=DR)
                            # exp((att+att_fei)/16): PSUM -> SBUF on ACT
                            nc.scalar.activation(expT[:, cj, :], attp[:], EXP,
                                                 scale=SCALE)
                            # bf16 column-sum accumulation on DVE (2x mode)
                            if cj == 1:
                                nc.vector.tensor_add(sumc[:], expT[:, 0, :],
                                                     expT[:, 1, :])
                            elif cj > 1:
                                nc.vector.tensor_add(sumc[:], sumc[:],
                                                     expT[:, cj, :])
                        if ob >= 0:
                            # score@v for block b-1, chunk cj (dep-free: its
                            # exps finished a block ago)
                            for nh in range(2):
                                nc.tensor.matmul(
                                    outp[nh][:],
                                    v_sb[:, ob * DF + cj * 128:
                                         ob * DF + (cj + 1) * 128],
                                    expO[:, cj, nh * 512:(nh + 1) * 512],
                                    start=(cj == 0), stop=(cj == 7))
                    if ob >= 0:
                        bs = ob * 128
                        bcastR = bcast_t.pop(ob)
                        outT = op.tile([128, 1024], BF16, tag="outT")
                        for nh in range(2):
                            nc.vector.tensor_mul(
                                outT[:, nh * 512:(nh + 1) * 512], outp[nh][:],
                                bcastR[:, nh * 512:(nh + 1) * 512])
                        # fc_out for this block: (128 rows, 512 dm)
                        fcp = psA.tile([128, 512], F32, tag="psA")
                        for c in range(CH):
                            nc.tensor.matmul(fcp[:],
                                             outT[:, c * 128:(c + 1) * 128],
                                             wo_sb[:, c, :],
                                             start=(c == 0), stop=(c == 7))
                        final = op.tile([128, 512], F32, tag="final")
                        nc.vector.tensor_copy(final[:], fcp[:])
                        nc.gpsimd.dma_start(out[bs:bs + 128, :], final[:])
                    if b < NB:
                        sum_block(b, sumc)

                if PROJ:
                    # consume kk/qq/v cheaply: copy slices out via gpsimd
                    junk = op.tile([128, 512], F32, tag="final")
                    nc.vector.tensor_copy(junk[:, 0:128],
                                          kk[:, 0, 0, 0:128])
                    nc.vector.tensor_copy(junk[:, 128:256],
                                          qq[:, 0, 0, 0:128])
                    nc.vector.tensor_copy(junk[:, 256:384],
                                          v_sb[:, 0:128])
                    for b in range(NB):
                        nc.gpsimd.dma_start(out[b * 128:(b + 1) * 128, :],
                                            junk[:])
                else:
                    for b in range(NB + 1):
                        att_out_block(b)

    nc.compile()
    return nc


def build_dma(loop_n=1):
    """DMA-only probe: all input loads + output stores, no compute."""
    nc = bacc.Bacc("TRN2", target_bir_lowering=False, debug=False)
    xT = nc.dram_tensor("xT", [DIN, R], BF16, kind="ExternalInput")
    xT8 = nc.dram_tensor("xT8", [DIN, R], F8, kind="ExternalInput")
    xkT8 = nc.dram_tensor("xkT8", [DIN, R], F8, kind="ExternalInput")
    wvt = nc.dram_tensor("wvt", [DIN, DF], BF16, kind="ExternalInput")
    w1_names = ["wkt8", "wqt8", "wkkt8", "wkqt8"]
    w1 = {n: nc.dram_tensor(n, [128, 2, 2, DF], F8, kind="ExternalInput")
          for n in w1_names}
    w2_names = ["wk2t8", "wq2t8", "wkft8", "wqft8"]
    w2 = {n: nc.dram_tensor(n, [128, 4, 2, DF], F8, kind="ExternalInput")
          for n in w2_names}
    woutt = nc.dram_tensor("woutt", [DF, DIN], BF16, kind="ExternalInput")
    ones = nc.dram_tensor("ones", [128, 128], BF16, kind="ExternalInput")
    bv = nc.dram_tensor("bv", [DF], F32, kind="ExternalInput")
    b2pack = nc.dram_tensor("b2pack", [128, 32], F32, kind="ExternalInput")
    out = nc.dram_tensor("out", [R, DIN], F32, kind="ExternalOutput")
    with tile.TileContext(nc) as tc:
        with (
            tc.tile_pool(name="xp", bufs=1) as xp,
            tc.tile_pool(name="wvp", bufs=4) as wvp,
            tc.tile_pool(name="w1p", bufs=4) as w1p,
            tc.tile_pool(name="w2p", bufs=4) as w2p,
            tc.tile_pool(name="wop", bufs=1) as wop,
            tc.tile_pool(name="bp", bufs=1) as bp,
            tc.tile_pool(name="op", bufs=1) as op,
        ):
            from contextlib import nullcontext
            loop_ctx = tc.For_i(0, loop_n, 1) if loop_n > 1 else nullcontext()
            with loop_ctx:
                xt_sb = xp.tile([128, 4, R], BF16, tag="xt")
                x8_sb = xp.tile([128, 4, R], F8, tag="x8")
                xk8_sb = xp.tile([128, 4, R], F8, tag="xk8")
                for kc in range(4):
                    nc.sync.dma_start(xt_sb[:, kc, :],
                                      xT[kc * 128:(kc + 1) * 128, :])
                    nc.sync.dma_start(x8_sb[:, kc, :],
                                      xT8[kc * 128:(kc + 1) * 128, :])
                    nc.sync.dma_start(xk8_sb[:, kc, :],
                                      xkT8[kc * 128:(kc + 1) * 128, :])
                for kc in range(4):
                    t = wvp.tile([128, DF], BF16, tag="wv")
                    nc.sync.dma_start(t[:], wvt[kc * 128:(kc + 1) * 128, :])
                for n in w1_names:
                    t = w1p.tile([128, 2, 2, DF], F8, tag="w1")
                    nc.sync.dma_start(t[:], w1[n][:])
                for n in w2_names:
                    t = w2p.tile([128, 4, 2, DF], F8, tag="w2")
                    nc.sync.dma_start(t[:], w2[n][:])
                wo_d = wop.tile([128, CH, DIN], BF16, tag="wo")
                nc.sync.dma_start(
                    wo_d[:], woutt.rearrange("(c p) j -> p c j", p=128))
                bvb = bp.tile([128, DF], F32, tag="bvb")
                nc.sync.dma_start(bvb[:], bass.AP(bv, 0, [[0, 128], [1, DF]]))
                b2p_sb = bp.tile([128, 32], F32, tag="b2p")
                nc.sync.dma_start(b2p_sb[:], b2pack[:])
                ones_sb = bp.tile([128, 128], BF16, tag="ones")
                nc.sync.dma_start(ones_sb[:], ones[:])
                fin_d = op.tile([128, 512], F32, tag="final")
                nc.gpsimd.memset(fin_d[:], 0.0)
                for b in range(NB):
                    nc.sync.dma_start(out[b * 128:(b + 1) * 128, :], fin_d[:])
    nc.compile()
    return nc


def prep_in_maps(inputs):
    import ml_dtypes
    NPBF = ml_dtypes.bfloat16
    NPF8 = ml_dtypes.float8_e4m3

    x = np.ascontiguousarray(inputs["x"], dtype=np.float32)
    xk = np.ascontiguousarray(inputs["x_knowledge"], dtype=np.float32)
    B, L, DM = x.shape
    x_flat = x.reshape(B * L, DM)
    xk_flat = xk.reshape(B * L, DM)
    f32 = np.float32

    def pack_w(name, npairs):
        # [p, pr, i, m] = W.T[(2*pr+i)*128+p, m], fp8
        WT = np.ascontiguousarray(np.asarray(inputs[name], f32).T)
        arr = WT.reshape(npairs, 2, 128, DF).transpose(2, 0, 1, 3)
        return np.ascontiguousarray(arr).astype(NPF8)

    def fold_b2(w2n, b2n, b1n):
        return (np.asarray(inputs[b2n], f32)
                + np.asarray(inputs[w2n], f32) @ np.asarray(inputs[b1n], f32))

    b2pack = np.zeros((128, 32), dtype=f32)
    for i, (w2n, b2n, b1n) in enumerate([
        ("Wk2", "bk2", "bk"), ("Wq2", "bq2", "bq"),
        ("Wkf", "bkf", "bkk"), ("Wqf", "bqf", "bkq"),
    ]):
        b2pack[:, i * 8:(i + 1) * 8] = fold_b2(w2n, b2n, b1n).reshape(8, 128).T

    shared = {
        "wvt": np.ascontiguousarray(np.asarray(inputs["Wv"], f32).T).astype(NPBF),
        "wkt8": pack_w("Wk", 2), "wqt8": pack_w("Wq", 2),
        "wkkt8": pack_w("Wkk", 2), "wkqt8": pack_w("Wkq", 2),
        "wk2t8": pack_w("Wk2", 4), "wq2t8": pack_w("Wq2", 4),
        "wkft8": pack_w("Wkf", 4), "wqft8": pack_w("Wqf", 4),
        "woutt": np.ascontiguousarray(np.asarray(inputs["Wout"], f32).T).astype(NPBF),
        "ones": np.ones((128, 128), dtype=NPBF),
        "bv": np.asarray(inputs["bv"], dtype=f32),
        "b2pack": b2pack,
    }
    in_maps = []
    for c in range(N_CORES):
        sl = slice(c * R, (c + 1) * R)
        m = dict(shared)
        xTc = np.ascontiguousarray(x_flat[sl].T)
        xkTc = np.ascontiguousarray(xk_flat[sl].T)
        m["xT"] = xTc.astype(NPBF)
        m["xT8"] = xTc.astype(NPF8)
        m["xkT8"] = xkTc.astype(NPF8)
        in_maps.append(m)
    return in_maps


def kernel(**inputs):
    if "nc" not in _CACHE:
        _CACHE["nc"] = build()
    nc = _CACHE["nc"]
    in_maps = prep_in_maps(inputs)
    B, L, DM = inputs["x"].shape
    f32 = np.float32

    res = run_bass_kernel_spmd(nc, in_maps, core_ids=list(range(N_CORES)))
    _CACHE["last_results"] = res
    out_flat = np.concatenate([res.results[c]["out"] for c in range(N_CORES)],
                              axis=0)
    out_flat = out_flat + np.asarray(inputs["bout"], dtype=f32)[None, :]
    return out_flat.reshape(B, L, DM).astype(np.float32)


if __name__ == "__main__":
    if "--compile-only" in sys.argv:
        import tempfile
        from concourse.bass_utils import compile_bass_kernel
        nc = build()
        print("bacc build OK; walrus-compiling...")
        print("OK:", compile_bass_kernel(nc, tempfile.mkdtemp()))
